# revision 61
# baseline (speedup 1.0000x reference)
"""BertSelfAttention on 8 Trainium2 NeuronCores (Bass/Tile).

Sharding: data-parallel over batch (B=2) x tensor-parallel over heads
(16 heads -> 4 groups of 4). Core c handles batch c//4, head group c%4,
holding column shards of Wq/Wk/Wv. No collectives.

v3 (final, ~181us vs 194us baseline; bf16 out): per-step pipeline where each of
the 128 (pass, kt) steps does scores -> exp -> ctx with the exp split
across two engines, ctx lagging two steps so the PE never waits on it:

  * exp split per score tile: ScalarE does one head's 512-col half
    (exact exp), DVE does the other via a Schraudolph bit-trick in ONE
    tensor_scalar: i16 = round(s*0.125*128*log2e + (127-sigma)*128)
    bitcast as bf16 (~3% max rel err on half the probability mass ->
    1.12e-2 total L2 err, inside the 2e-2 gate). Alternates per step.
  * per-engine score psum rings (sscA/sscD) + per-engine es tiles:
    a shared ssc tile or shared es tile couples the two engines' WAR/
    WAW chains and was the pace-setter. Steady state is now bound by
    the ACT es-op cycle (~690ns op + ~190ns sequencer) -> ~940ns/step.
  * streamed head: xT arrives block-major ([128, 4 blk, 8 kt, 512])
    kt-sliced 3 ways across the sync/scalar/gpsimd DMA queues in
    global priority order (a single queue sustains only ~140 GB/s of
    the ~320 GB/s aggregate; the gpsimd SWDGE queue flood-posts and
    outraces the others, so it carries only early-priority slices).
    m=0 K/Q projections run per 512-seq block as its DMA lands.
  * GPSIMD cannot touch PSUM (BIR verifier) so evacuations stay on
    ACT (K-m0, ctx-out hh0) and DVE (Q-m0, V, m1, ctx-out hh1).
  * fillers (m0 blocks, V tiles, m1 quarter-sweeps) are deadline-
    scheduled units; per step: filler MMs, scores+exp, 1-step-old out
    copies, ctx(i-2), filler evacs. Evacs trail their MMs so in-order
    engine queues never head-of-line block the latency-critical exp.
  * warm-up matmuls use a full 128-row stationary: the PE p-state
    governor tracks utilization, and narrow warmups never ramp the
    clock past 1.2GHz (full rate is 2.4GHz, reached after ~3-10us of
    dense full-width work; any multi-us stall drops it back).

PSUM (8 banks):
  tag "sscA" 2x[128,512] (2): ACT-half score tiles, double buffered
  tag "sscD" 2x[128,512] (2): DVE-half score tiles, double buffered
  tag "ctx"  2x[65,512]  (2): ctx+denominator accumulators (hh pair)
  tag "a"    2x[128,512] (2): m0 blocks / V-proj / m1 sweeps / warmup

Per head the ctx stationary is [V_h | ones] (65 cols): PSUM row 65 of
each ctx tile accumulates the softmax denominators for free. Host
unshards: out[b, :, g*256 + 64h + r] = (ctx_h / sums_h).T
"""

import sys

sys.path.insert(0, "/opt/trn_rl_repo")

import numpy as np

try:
    import ml_dtypes

    _BF16 = ml_dtypes.bfloat16
except ImportError:  # pragma: no cover
    import jax.numpy as jnp

    _BF16 = jnp.bfloat16

import concourse.bass as bass
import concourse.mybir as mybir
import concourse.tile as tile
from concourse import bacc
from concourse import bass_utils as _bass_utils
from concourse.bass_utils import run_bass_kernel_spmd

F32 = mybir.dt.float32
BF16 = mybir.dt.bfloat16
I16 = mybir.dt.int16

HIDDEN = 1024
NUM_HEADS = 16
HEAD = 64
B, S = 2, 2048
N_CORES = 8
GROUPS = 4                      # head groups (tensor parallel)
HG = NUM_HEADS // GROUPS        # heads per group = 4
DG = HG * HEAD                  # 256 cols per group
KT_TILES = HIDDEN // 128        # 8 contraction tiles for projections
ST_TILES = S // 128             # 16 sequence tiles
QC = 512                        # q chunk width (one pass = one chunk)
N_QC = S // QC                  # 4
NBLK = 4                        # xT streaming blocks of 512 seq positions
VAUG = HG * (HEAD + 1)          # 260: [V_h | ones] per head

# Schraudolph fast-exp constants (bf16 exponent domain, minimax sigma).
# es = bitcast_bf16(int16(round(s * EXP_MUL + EXP_ADD))) ~= exp(s / 8)
_LOG2E = 1.4426950408889634
EXP_MUL = 0.125 * 128.0 * _LOG2E
EXP_ADD = (127.0 - 0.04303) * 128.0


def _build_kernel():
    nc = bacc.Bacc("TRN2")

    # xT block-major: xTb[p, b, kt, s] = x[b*512+s, kt*128+p]; each
    # [:, b] slice is 8KB contiguous per partition on both sides.
    xTb = nc.dram_tensor("xTb", [128, NBLK, KT_TILES, QC], BF16,
                         kind="ExternalInput")
    # wqk[p, m, kt, :] = [Wq_m | Wk_m][kt*128+p, :] (partition-major
    # SBUF image; 4KB per-partition DMA segments).
    wqk = nc.dram_tensor(
        "wqk", [128, 2, KT_TILES, DG], BF16, kind="ExternalInput"
    )
    # wv pre-augmented (per head 64 cols + zero col), partition-major.
    wv = nc.dram_tensor(
        "wv", [128, KT_TILES, VAUG], BF16, kind="ExternalInput"
    )
    # per-partition bias cols: bq[0:128], bq[128:], bk[0:128], bk[128:]
    bqk = nc.dram_tensor("bqk", [128, 4], F32, kind="ExternalInput")
    # bv interleaved with 1.0 at each head's ones column [1, 260]
    bv_aug = nc.dram_tensor("bv_aug", [1, VAUG], BF16, kind="ExternalInput")
    out_raw = nc.dram_tensor("out_raw", [VAUG, S], BF16, kind="ExternalOutput")

    with tile.TileContext(nc) as tc:
        with (
            tc.tile_pool(name="consts", bufs=1) as consts,
            tc.tile_pool(name="esp", bufs=4) as esp,
            tc.tile_pool(name="outp", bufs=4) as outp,
            tc.tile_pool(name="ps", bufs=2, space="PSUM") as ps,
        ):
            # ---- loads. A single DGE queue sustains only ~140 GB/s, so
            # the critical tensors are spread across four engine queues
            # (each SBUF tile written by exactly one queue — two queues
            # on one tile wedges the device). Block 0 rides the
            # otherwise-idle Pool queue so pass (0,0) can start earliest.
            # HBM aggregate is ~358 GB/s shared by the 3 DMA-capable
            # queues (SP/ACT/Pool), each queue alone sustaining only
            # ~140 GB/s. Every large tensor is therefore kt-sliced
            # 3 ways (slices are separate SBUF tiles: two queues
            # writing one tile wedges the device) and the slices are
            # enqueued in global priority order, so each tensor
            # completes at full aggregate bandwidth before the next:
            # wqk0 -> blk0 -> wv -> blk1 -> blk2 -> blk3 -> wqk1.
            bqk_sb = consts.tile([128, 4], F32)
            bvaug_sb = consts.tile([1, VAUG], BF16)
            ones_sb = consts.tile([1, QC], BF16)
            nc.vector.memset(ones_sb[:], 1.0)
            # full-width warm-up operand: the PE p-state governor tracks
            # utilization, so warm-up matmuls need a 128-row stationary
            # (a [1,128] stationary keeps the array at idle-level util
            # and never ramps the clock).
            warm_sb = consts.tile([128, 384], BF16)
            nc.vector.memset(warm_sb[:], 0.0)

            def _splitk(src, inner, name, parts):
                """DMA src [128, KT_TILES, inner] as kt-slices
                (lo, hi, queue); returns a kt-indexable accessor."""
                tiles = []
                for qi, (lo, hi, q) in enumerate(parts):
                    t = consts.tile([128, hi - lo, inner], BF16,
                                    name=f"{name}{qi}")
                    q.dma_start(t[:], src[:, lo:hi])
                    tiles.append((lo, hi, t))

                def at(kt):
                    for lo, hi, t in tiles:
                        if lo <= kt < hi:
                            return t[:, kt - lo]
                return at

            # The gpsimd SWDGE queue flood-posts its ring and its
            # transfers outrace the issue-serialized sync/scalar HWDGE
            # queues (whose first ~0.6MB crawls until ~17.6us no matter
            # the ordering). So the entire Kb0-critical set (wqk0+xt0)
            # rides gpsimd as its first entries; sync/scalar carry the
            # later tensors two-way in global priority order:
            # wv -> xt1 -> xt2 -> xt3 -> wqk1.
            _qs, _qc, _qg = nc.sync, nc.scalar, nc.gpsimd
            _p3 = lambda: ((0, 3, _qs), (3, 6, _qc), (6, 8, _qg))
            _p2 = lambda: ((0, 4, _qs), (4, 8, _qc))
            # wqk0 (0.5MB) rides the fast-flooding gpsimd queue whole
            # (~11us); xt0 halves are sync/scalar's FIRST entries so
            # Kb0 beats the ~17.6us HWDGE first-entries wall.
            wqk0_sb = consts.tile([128, KT_TILES, DG], BF16)
            nc.gpsimd.dma_start(wqk0_sb[:], wqk[:, 0])
            wqk0_kt = lambda kt: wqk0_sb[:, kt]
            nc.gpsimd.dma_start(bqk_sb[:], bqk[:])
            xt_b0 = _splitk(xTb[:, 0], QC, "xt0", _p2())
            wv_kt = _splitk(wv, VAUG, "wv", _p3())
            nc.gpsimd.dma_start(bvaug_sb[:], bv_aug[:])
            xt_b1 = _splitk(xTb[:, 1], QC, "xt1", _p3())
            xt_b2 = _splitk(xTb[:, 2], QC, "xt2", _p3())
            xt_b3 = _splitk(xTb[:, 3], QC, "xt3", _p3())
            wqk1_kt = _splitk(wqk[:, 1], DG, "wqk1", _p2())
            xts = (xt_b0, xt_b1, xt_b2, xt_b3)

            QT_sb = consts.tile([128, 2, S], BF16)
            KT_sb = consts.tile([128, 2, S], BF16)
            v_sb = consts.tile([128, ST_TILES, VAUG], BF16)

            # Filler units are (matmul-emit, evac-emit) pairs. Within a
            # step the emission order is: filler MMs, scores+exp,
            # ctx(i-1), filler evacs. Engine queues are in-order, so an
            # evac whose producing matmuls haven't retired would
            # head-of-line block the latency-critical exp op behind it;
            # with MMs at the head of the same step's TE queue, the evac
            # deps are long satisfied by the time the evac is queued.
            _acc = {}

            # ---- m=0 projection for one 512-seq block ----
            # wqk m0 layout per kt: Q cols 0:128, K cols 128:256.
            def m0_mm(b, which, warm=False, kts=range(KT_TILES)):
                col0 = 0 if which == "q" else 128
                if 0 in kts:
                    acc = ps.tile([128, QC], F32, tag="a",
                                  name=f"m0{which}{b}")
                    _acc[("m0", which, b)] = acc
                else:
                    acc = _acc[("m0", which, b)]
                if warm:
                    # p-state warm-up: hold the PE busy at FULL array
                    # utilization through the ~11us DMA load phase so
                    # the clock is ramped when real work arrives;
                    # garbage is erased by kt=0's start=True.
                    # count sized to span until block 0's last DMA slice
                    # lands (~15.5us): the PE must never idle before
                    # Kb0 or the p-state drops for most of pass 0.
                    for _ in range(41):
                        nc.tensor.matmul(
                            acc[:, 0:256], warm_sb[:, 0:128],
                            warm_sb[:, 128:384],
                            start=True, stop=True,
                        )
                for kt in kts:
                    nc.tensor.matmul(
                        acc[:],
                        wqk0_kt(kt)[:, col0:col0 + 128],
                        xts[b](kt),
                        start=(kt == 0), stop=(kt == KT_TILES - 1),
                    )

            def m0_evac(b, which):
                acc = _acc.pop(("m0", which, b))
                if which == "k":
                    nc.scalar.activation(
                        KT_sb[:, 0, b * QC:(b + 1) * QC], acc[:],
                        mybir.ActivationFunctionType.Identity,
                        bias=bqk_sb[:, 2:3],
                    )
                else:
                    nc.vector.tensor_scalar_add(
                        QT_sb[:, 0, b * QC:(b + 1) * QC], acc[:],
                        bqk_sb[:, 0:1],
                    )

            def v_mm(st):
                psv = ps.tile([128, QC], F32, tag="a", name="psv")
                _acc[("v", st)] = psv
                blk, sub = st // 4, st % 4
                for kt in range(KT_TILES):
                    nc.tensor.matmul(
                        psv[:, 0:VAUG],
                        xts[blk](kt)[:, sub * 128:(sub + 1) * 128],
                        wv_kt(kt),
                        start=(kt == 0), stop=False,
                    )
                nc.tensor.matmul(
                    psv[:, 0:VAUG], ones_sb[:, 0:128], bvaug_sb[:, :],
                    start=False, stop=True,
                )

            def v_evac(st):
                psv = _acc.pop(("v", st))
                nc.vector.tensor_copy(out=v_sb[:, st, :], in_=psv[:, 0:VAUG])

            # m=1 projection: 8-kt sweep split into 2-kt quarters
            # spread over 4 steps (an 8-matmul burst in one step starves
            # the exp pipeline for ~2us).
            def m1_mm(wcol, sc, quarter):
                if quarter == 0:
                    acc = ps.tile([128, QC], F32, tag="a",
                                  name=f"m1_{wcol}_{sc}")
                    _acc[("m1", wcol, sc)] = acc
                else:
                    acc = _acc[("m1", wcol, sc)]
                for kt in range(quarter * 2, quarter * 2 + 2):
                    nc.tensor.matmul(
                        acc[:],
                        wqk1_kt(kt)[:, wcol:wcol + 128],
                        xts[sc](kt),
                        start=(kt == 0), stop=(kt == KT_TILES - 1),
                    )

            def m1_evac(dst_sb, wcol, bcol, sc):
                acc = _acc.pop(("m1", wcol, sc))
                nc.vector.tensor_scalar_add(
                    dst_sb[:, 1, sc * QC:(sc + 1) * QC], acc[:],
                    bqk_sb[:, bcol:bcol + 1],
                )

            # ---- head: warmup + block-0 m0 only; scores(0) fires as
            # early as possible, V0/V1 become step-1 units (ctx(0) is
            # emitted at step 2 under the lag-2 pipeline).
            m0_mm(0, "k", warm=True)
            m0_evac(0, "k")
            m0_mm(0, "q")
            m0_evac(0, "q")

            # ---- filler schedule: {step: [(mm_fn, evac_fn), ...]} ----
            # At most 2 units per step (tag "a" has 2 slots; a unit's
            # psum lives only phase1->phase4 of its step).
            F = {}

            def unit(step, mm, evac):
                F.setdefault(step, []).append((mm, evac))

            def v_unit(step, st):
                unit(step, lambda: v_mm(st), lambda: v_evac(st))

            # V(st) must land before ctx(st) at step st+2; K-m0 block b
            # before scores kt=4b at step 4b; Q-m0 block j before pass
            # (0,j) at step 16j.
            def m0_halves(s0, b, which):
                unit(s0, lambda: m0_mm(b, which, kts=range(0, 4)), None)
                unit(s0 + 1, lambda: m0_mm(b, which, kts=range(4, 8)),
                     lambda: m0_evac(b, which))

            # Slot discipline on tag "a" (2 slots): each 2-step m0-half
            # holder allocates with exactly ONE transient V unit beside
            # it (other slot) and nothing allocates during its second
            # step, so evacs always precede reallocation.
            v_unit(1, 0)
            v_unit(1, 1)
            m0_halves(2, 1, "k")   # live 2-3
            v_unit(2, 2)
            v_unit(4, 3)
            v_unit(4, 4)
            m0_halves(5, 2, "k")   # live 5-6
            v_unit(5, 5)
            v_unit(7, 6)
            v_unit(7, 7)
            m0_halves(8, 3, "k")   # live 8-9
            v_unit(8, 8)
            v_unit(10, 9)
            v_unit(10, 10)
            m0_halves(11, 1, "q")  # live 11-12
            v_unit(11, 11)
            v_unit(13, 12)
            v_unit(13, 13)
            v_unit(14, 14)
            v_unit(14, 15)
            # Qb2/Qb3 split into 4-kt halves (an 8-MM burst in one step
            # starves the exp pipeline for ~2us)
            unit(30, lambda: m0_mm(2, "q", kts=range(0, 4)), None)
            unit(31, lambda: m0_mm(2, "q", kts=range(4, 8)),
                 lambda: m0_evac(2, "q"))
            unit(42, lambda: m0_mm(3, "q", kts=range(0, 4)), None)
            unit(43, lambda: m0_mm(3, "q", kts=range(4, 8)),
                 lambda: m0_evac(3, "q"))
            # m=1 sweeps: K all before pass (1,0) at step 64; Q chunk sc
            # before pass (1,sc) at step 64+16sc. Quarters at dl..dl+3,
            # evac with the last quarter.
            for i, dl in enumerate((20, 28, 38, 48)):
                for q in range(4):
                    unit(dl + q, lambda s=i, q=q: m1_mm(128, s, q),
                         (lambda s=i: m1_evac(KT_sb, 128, 3, s))
                         if q == 3 else None)
            for i, dl in enumerate((54, 70, 86, 100)):
                for q in range(4):
                    unit(dl + q, lambda s=i, q=q: m1_mm(0, s, q),
                         (lambda s=i: m1_evac(QT_sb, 0, 1, s))
                         if q == 3 else None)

            # ---- attention: 8 passes x 16 kt steps, lag-1 pipeline ----
            passes = [(p, j) for p in range(2) for j in range(N_QC)]
            steps = [(pi, kt) for pi in range(len(passes))
                     for kt in range(ST_TILES)]
            n_steps = len(steps)

            es_tiles = {}
            ctx_tiles = {}

            def emit_scores(i):
                pi, kt = steps[i]
                p, j = passes[pi]
                # Per-engine score/es tiles: separate psum rings (sscA /
                # sscD) keep each half's WAR chain independent, and
                # separate es tiles avoid a false WAW between the ACT
                # and DVE halves.
                hh_act = i % 2          # exact-exp half alternates
                hh_dve = 1 - hh_act
                sA = ps.tile([128, QC], F32, tag="sscA", name="sscA")
                sD = ps.tile([128, QC], F32, tag="sscD", name="sscD")
                for hh, st in ((hh_act, sA), (hh_dve, sD)):
                    rows = slice(hh * 64, hh * 64 + 64)
                    nc.tensor.matmul(
                        st[:],
                        KT_sb[rows, p, kt * 128:(kt + 1) * 128],
                        QT_sb[rows, p, j * QC:(j + 1) * QC],
                        start=True, stop=True,
                    )
                esA = esp.tile([128, QC], BF16, tag="esA", name="esA")
                esD = esp.tile([128, QC], I16, tag="esD", name="esD")
                nc.scalar.activation(
                    esA[:], sA[:],
                    mybir.ActivationFunctionType.Exp, scale=0.125,
                )
                nc.vector.tensor_scalar(
                    esD[:], sD[:],
                    EXP_MUL, EXP_ADD,
                    mybir.AluOpType.mult, mybir.AluOpType.add,
                )
                es_tiles[i] = {hh_act: esA[:], hh_dve: esD[:].bitcast(BF16)}

            pending_out = []

            def emit_ctx(i):
                pi, kt = steps[i]
                p, j = passes[pi]
                if kt == 0:
                    for hh in range(2):
                        ctx_tiles[(pi, hh)] = ps.tile(
                            [65, QC], F32, tag="ctx", name="ctx"
                        )
                es = es_tiles.pop(i)
                for hh in range(2):
                    h = 2 * p + hh
                    nc.tensor.matmul(
                        ctx_tiles[(pi, hh)][:],
                        v_sb[:, kt, h * 65:(h + 1) * 65],
                        es[hh],
                        start=(kt == 0), stop=(kt == ST_TILES - 1),
                    )
                if kt == ST_TILES - 1:
                    pending_out.append((i, pi,
                                        ctx_tiles.pop((pi, 0)),
                                        ctx_tiles.pop((pi, 1))))

            def flush_out(before_step=None):
                # pass-boundary out evac: hh0 on ACT, hh1 on DVE, run in
                # parallel; emitted at the TAIL of the following step's
                # queues (after its exp ops) so the ctx-stop matmul has
                # retired and the copies neither stall the exp ops nor
                # open a bubble.
                while pending_out and (before_step is None
                                       or pending_out[0][0] < before_step):
                    _, pi, c0, c1 = pending_out.pop(0)
                    p, j = passes[pi]
                    for hh, cpsum in ((0, c0), (1, c1)):
                        h = 2 * p + hh
                        ctx_sb = outp.tile([65, QC], BF16, tag="o",
                                           name="ctx_sb")
                        if hh == 0:
                            nc.scalar.copy(out=ctx_sb[:], in_=cpsum[:])
                        else:
                            nc.vector.tensor_copy(out=ctx_sb[:],
                                                  in_=cpsum[:])
                        # alternate queues so the last pass's outputs
                        # drain in parallel instead of serializing
                        q = (nc.sync, nc.scalar)[hh]
                        q.dma_start(
                            out_raw[h * 65:(h + 1) * 65,
                                    j * QC:(j + 1) * QC],
                            ctx_sb[:],
                        )

            for i in range(n_steps):
                units = F.get(i, ())
                for mm, _ in units:          # phase 1: filler matmuls
                    if mm:
                        mm()
                emit_scores(i)               # phase 2: scores + exp
                # out-copies queue behind this step's exp ops; must be
                # emitted BEFORE emit_ctx's kt=0 alloc reuses the psum.
                flush_out(before_step=i)
                if i > 1:
                    emit_ctx(i - 2)          # phase 3 (lag-2: the
                    # scores->exp->ctx chain never stalls the PE)
                for _, evac in units:        # phase 4: filler evacs
                    if evac:
                        evac()
            emit_ctx(n_steps - 2)
            emit_ctx(n_steps - 1)
            flush_out()
    nc.compile()
    return nc


_NC_CACHE = None


def _get_nc():
    global _NC_CACHE
    if _NC_CACHE is None:
        _NC_CACHE = _build_kernel()
    return _NC_CACHE


def _prep_core_inputs(hidden_states, Wq, bq, Wk, bk, Wv, bv):
    """Host-side sharding: returns list of 8 in_maps (bf16 pre-cast)."""
    # xT [1024, 2048] -> block-major [128 p, 4 blk, 8 kt, 512 s]
    xTbs = [
        np.ascontiguousarray(
            hidden_states[b].T.reshape(KT_TILES, 128, NBLK, QC)
            .transpose(1, 2, 0, 3)
        ).astype(_BF16)
        for b in range(B)
    ]
    in_maps = []
    for c in range(N_CORES):
        b, g = divmod(c, GROUPS)
        cs = slice(g * DG, (g + 1) * DG)
        wq_g = Wq[:, cs]
        wk_g = Wk[:, cs]
        wv_g = Wv[:, cs]
        bq_g, bk_g, bv_g = bq[cs], bk[cs], bv[cs]

        wv_aug = np.zeros((HIDDEN, VAUG), dtype=np.float32)
        bv_aug = np.zeros((1, VAUG), dtype=np.float32)
        for h in range(HG):
            wv_aug[:, h * 65:h * 65 + 64] = wv_g[:, h * 64:(h + 1) * 64]
            bv_aug[0, h * 65:h * 65 + 64] = bv_g[h * 64:(h + 1) * 64]
            bv_aug[0, h * 65 + 64] = 1.0

        bqk = np.stack(
            [bq_g[:128], bq_g[128:], bk_g[:128], bk_g[128:]], axis=1
        ).astype(np.float32)

        in_maps.append(
            {
                "xTb": xTbs[b],
                # partition-major SBUF image [128, 2, 8, 256]
                "wqk": np.ascontiguousarray(
                    np.stack([
                        np.concatenate(
                            [wq_g[:, m * 128:(m + 1) * 128],
                             wk_g[:, m * 128:(m + 1) * 128]], 1
                        ).reshape(KT_TILES, 128, DG).transpose(1, 0, 2)
                        for m in range(2)
                    ], axis=1)
                ).astype(_BF16),
                # partition-major SBUF image [128, 8, 260]
                "wv": np.ascontiguousarray(
                    wv_aug.reshape(KT_TILES, 128, VAUG).transpose(1, 0, 2)
                ).astype(_BF16),
                "bqk": np.ascontiguousarray(bqk),
                "bv_aug": bv_aug.astype(_BF16),
            }
        )
    return in_maps


def _unshard(results):
    out = np.empty((B, S, HIDDEN), dtype=np.float32)
    for c in range(N_CORES):
        b, g = divmod(c, GROUPS)
        raw = np.asarray(results[c]["out_raw"], dtype=np.float32)
        for h in range(HG):
            ctx = raw[h * 65:h * 65 + 64]          # [64, S]
            sums = raw[h * 65 + 64]                # [S]
            col0 = g * DG + h * HEAD
            out[b, :, col0:col0 + HEAD] = (ctx / sums).T
    return out


def kernel(**inputs):
    inputs = {k: np.asarray(v, dtype=np.float32) for k, v in inputs.items()}
    nc = _get_nc()
    in_maps = _prep_core_inputs(**inputs)
    res = run_bass_kernel_spmd(nc, in_maps, core_ids=list(range(N_CORES)))
    return _unshard(res.results)


if __name__ == "__main__":
    rng = np.random.default_rng(0)
    scale = 1.0 / np.sqrt(HIDDEN)
    ins = {
        "hidden_states": rng.standard_normal((B, S, HIDDEN), dtype=np.float32),
        "Wq": rng.standard_normal((HIDDEN, HIDDEN), dtype=np.float32) * scale,
        "bq": rng.standard_normal(HIDDEN, dtype=np.float32) * 0.01,
        "Wk": rng.standard_normal((HIDDEN, HIDDEN), dtype=np.float32) * scale,
        "bk": rng.standard_normal(HIDDEN, dtype=np.float32) * 0.01,
        "Wv": rng.standard_normal((HIDDEN, HIDDEN), dtype=np.float32) * scale,
        "bv": rng.standard_normal(HIDDEN, dtype=np.float32) * 0.01,
    }
    out = kernel(**ins)

    def ref(x, Wq, bq, Wk, bk, Wv, bv):
        q = (x @ Wq + bq).reshape(B, S, NUM_HEADS, HEAD).transpose(0, 2, 1, 3)
        k = (x @ Wk + bk).reshape(B, S, NUM_HEADS, HEAD).transpose(0, 2, 1, 3)
        v = (x @ Wv + bv).reshape(B, S, NUM_HEADS, HEAD).transpose(0, 2, 1, 3)
        s = np.einsum("bhqd,bhkd->bhqk", q, k) / np.sqrt(HEAD)
        s = s - s.max(-1, keepdims=True)
        p = np.exp(s)
        p /= p.sum(-1, keepdims=True)
        c = np.einsum("bhqk,bhkd->bhqd", p, v)
        return c.transpose(0, 2, 1, 3).reshape(B, S, HIDDEN)

    exp = ref(
        ins["hidden_states"].astype(np.float64),
        ins["Wq"].astype(np.float64), ins["bq"].astype(np.float64),
        ins["Wk"].astype(np.float64), ins["bk"].astype(np.float64),
        ins["Wv"].astype(np.float64), ins["bv"].astype(np.float64),
    )
    print("L2 rel err:", np.linalg.norm(out - exp) / np.linalg.norm(exp))
    print("max abs err:", np.abs(out - exp).max())


# revision 63
# speedup vs baseline: 1.1887x; 1.1887x over previous
"""BertSelfAttention on 8 Trainium2 NeuronCores (Bass/Tile).

Sharding: data-parallel over batch (B=2) x tensor-parallel over heads
(16 heads -> 4 groups of 4). Core c handles batch c//4, head group c%4,
holding column shards of Wq/Wk/Wv. No collectives.

v3 (final, ~181us vs 194us baseline; bf16 out): per-step pipeline where each of
the 128 (pass, kt) steps does scores -> exp -> ctx with the exp split
across two engines, ctx lagging two steps so the PE never waits on it:

  * exp split per score tile: ScalarE does one head's 512-col half
    (exact exp), DVE does the other via a Schraudolph bit-trick in ONE
    tensor_scalar: i16 = round(s*0.125*128*log2e + (127-sigma)*128)
    bitcast as bf16 (~3% max rel err on half the probability mass ->
    1.12e-2 total L2 err, inside the 2e-2 gate). Alternates per step.
  * per-engine score psum rings (sscA/sscD) + per-engine es tiles:
    a shared ssc tile or shared es tile couples the two engines' WAR/
    WAW chains and was the pace-setter. Steady state is now bound by
    the ACT es-op cycle (~690ns op + ~190ns sequencer) -> ~940ns/step.
  * streamed head: xT arrives block-major ([128, 4 blk, 8 kt, 512])
    kt-sliced 3 ways across the sync/scalar/gpsimd DMA queues in
    global priority order (a single queue sustains only ~140 GB/s of
    the ~320 GB/s aggregate; the gpsimd SWDGE queue flood-posts and
    outraces the others, so it carries only early-priority slices).
    m=0 K/Q projections run per 512-seq block as its DMA lands.
  * GPSIMD cannot touch PSUM (BIR verifier) so evacuations stay on
    ACT (K-m0, ctx-out hh0) and DVE (Q-m0, V, m1, ctx-out hh1).
  * fillers (m0 blocks, V tiles, m1 quarter-sweeps) are deadline-
    scheduled units; per step: filler MMs, scores+exp, 1-step-old out
    copies, ctx(i-2), filler evacs. Evacs trail their MMs so in-order
    engine queues never head-of-line block the latency-critical exp.
  * warm-up matmuls use a full 128-row stationary: the PE p-state
    governor tracks utilization, and narrow warmups never ramp the
    clock past 1.2GHz (full rate is 2.4GHz, reached after ~3-10us of
    dense full-width work; any multi-us stall drops it back).

PSUM (8 banks):
  tag "sscA" 2x[128,512] (2): ACT-half score tiles, double buffered
  tag "sscD" 2x[128,512] (2): DVE-half score tiles, double buffered
  tag "ctx"  2x[65,512]  (2): ctx+denominator accumulators (hh pair)
  tag "a"    2x[128,512] (2): m0 blocks / V-proj / m1 sweeps / warmup

Per head the ctx stationary is [V_h | ones] (65 cols): PSUM row 65 of
each ctx tile accumulates the softmax denominators for free. Host
unshards: out[b, :, g*256 + 64h + r] = (ctx_h / sums_h).T
"""

import sys

sys.path.insert(0, "/opt/trn_rl_repo")

import numpy as np

try:
    import ml_dtypes

    _BF16 = ml_dtypes.bfloat16
except ImportError:  # pragma: no cover
    import jax.numpy as jnp

    _BF16 = jnp.bfloat16

import concourse.bass as bass
import concourse.mybir as mybir
import concourse.tile as tile
from concourse import bacc
from concourse import bass_utils as _bass_utils
from concourse.bass_utils import run_bass_kernel_spmd

F32 = mybir.dt.float32
BF16 = mybir.dt.bfloat16
I16 = mybir.dt.int16

HIDDEN = 1024
NUM_HEADS = 16
HEAD = 64
B, S = 2, 2048
N_CORES = 8
GROUPS = 4                      # head groups (tensor parallel)
HG = NUM_HEADS // GROUPS        # heads per group = 4
DG = HG * HEAD                  # 256 cols per group
KT_TILES = HIDDEN // 128        # 8 contraction tiles for projections
ST_TILES = S // 128             # 16 sequence tiles
QC = 512                        # q chunk width (one pass = one chunk)
N_QC = S // QC                  # 4
NBLK = 4                        # xT streaming blocks of 512 seq positions
VAUG = HG * (HEAD + 1)          # 260: [V_h | ones] per head

# Schraudolph fast-exp constants (bf16 exponent domain, minimax sigma).
# es = bitcast_bf16(int16(round(s * EXP_MUL + EXP_ADD))) ~= exp(s / 8)
_LOG2E = 1.4426950408889634
EXP_MUL = 0.125 * 128.0 * _LOG2E
EXP_ADD = (127.0 - 0.04303) * 128.0


def _build_kernel():
    nc = bacc.Bacc("TRN2")

    # xT block-major: xTb[p, b, kt, s] = x[b*512+s, kt*128+p]; each
    # [:, b] slice is 8KB contiguous per partition on both sides.
    xTb = nc.dram_tensor("xTb", [128, NBLK, KT_TILES, QC], BF16,
                         kind="ExternalInput")
    # wqk[p, m, kt, :] = [Wq_m | Wk_m][kt*128+p, :] (partition-major
    # SBUF image; 4KB per-partition DMA segments).
    wqk = nc.dram_tensor(
        "wqk", [128, 2, KT_TILES, DG], BF16, kind="ExternalInput"
    )
    # wv pre-augmented (per head 64 cols + zero col), partition-major.
    wv = nc.dram_tensor(
        "wv", [128, KT_TILES, VAUG], BF16, kind="ExternalInput"
    )
    # per-partition bias cols: bq[0:128], bq[128:], bk[0:128], bk[128:]
    bqk = nc.dram_tensor("bqk", [128, 4], F32, kind="ExternalInput")
    # bv interleaved with 1.0 at each head's ones column [1, 260]
    bv_aug = nc.dram_tensor("bv_aug", [1, VAUG], BF16, kind="ExternalInput")
    out_raw = nc.dram_tensor("out_raw", [VAUG, S], BF16, kind="ExternalOutput")

    with tile.TileContext(nc) as tc:
        with (
            tc.tile_pool(name="consts", bufs=1) as consts,
            tc.tile_pool(name="esp", bufs=4) as esp,
            tc.tile_pool(name="outp", bufs=4) as outp,
            tc.tile_pool(name="ps", bufs=2, space="PSUM") as ps,
        ):
            # ---- loads. A single DGE queue sustains only ~140 GB/s, so
            # the critical tensors are spread across four engine queues
            # (each SBUF tile written by exactly one queue — two queues
            # on one tile wedges the device). Block 0 rides the
            # otherwise-idle Pool queue so pass (0,0) can start earliest.
            # HBM aggregate is ~358 GB/s shared by the 3 DMA-capable
            # queues (SP/ACT/Pool), each queue alone sustaining only
            # ~140 GB/s. Every large tensor is therefore kt-sliced
            # 3 ways (slices are separate SBUF tiles: two queues
            # writing one tile wedges the device) and the slices are
            # enqueued in global priority order, so each tensor
            # completes at full aggregate bandwidth before the next:
            # wqk0 -> blk0 -> wv -> blk1 -> blk2 -> blk3 -> wqk1.
            bqk_sb = consts.tile([128, 4], F32)
            bvaug_sb = consts.tile([1, VAUG], BF16)
            ones_sb = consts.tile([1, QC], BF16)
            nc.vector.memset(ones_sb[:], 1.0)
            # full-width warm-up operand: the PE p-state governor tracks
            # utilization, so warm-up matmuls need a 128-row stationary
            # (a [1,128] stationary keeps the array at idle-level util
            # and never ramps the clock).
            warm_sb = consts.tile([128, 384], BF16)
            nc.vector.memset(warm_sb[:], 0.0)

            def _splitk(src, inner, name, parts):
                """DMA src [128, KT_TILES, inner] as kt-slices
                (lo, hi, queue); returns a kt-indexable accessor."""
                tiles = []
                for qi, (lo, hi, q) in enumerate(parts):
                    t = consts.tile([128, hi - lo, inner], BF16,
                                    name=f"{name}{qi}")
                    q.dma_start(t[:], src[:, lo:hi])
                    tiles.append((lo, hi, t))

                def at(kt):
                    for lo, hi, t in tiles:
                        if lo <= kt < hi:
                            return t[:, kt - lo]
                return at

            # The gpsimd SWDGE queue flood-posts its ring and its
            # transfers outrace the issue-serialized sync/scalar HWDGE
            # queues (whose first ~0.6MB crawls until ~17.6us no matter
            # the ordering). So the entire Kb0-critical set (wqk0+xt0)
            # rides gpsimd as its first entries; sync/scalar carry the
            # later tensors two-way in global priority order:
            # wv -> xt1 -> xt2 -> xt3 -> wqk1.
            _qs, _qc, _qg = nc.sync, nc.scalar, nc.gpsimd
            _p3 = lambda: ((0, 3, _qs), (3, 6, _qc), (6, 8, _qg))
            _p2 = lambda: ((0, 4, _qs), (4, 8, _qc))
            # wqk0 (0.5MB) rides the fast-flooding gpsimd queue whole
            # (~11us); xt0 halves are sync/scalar's FIRST entries so
            # Kb0 beats the ~17.6us HWDGE first-entries wall.
            wqk0_sb = consts.tile([128, KT_TILES, DG], BF16)
            nc.gpsimd.dma_start(wqk0_sb[:], wqk[:, 0])
            wqk0_kt = lambda kt: wqk0_sb[:, kt]
            xt_b0 = _splitk(xTb[:, 0], QC, "xt0", _p2())
            nc.sync.dma_start(bqk_sb[:], bqk[:])
            wv_kt = _splitk(wv, VAUG, "wv", _p3())
            nc.gpsimd.dma_start(bvaug_sb[:], bv_aug[:])
            xt_b1 = _splitk(xTb[:, 1], QC, "xt1", _p3())
            xt_b2 = _splitk(xTb[:, 2], QC, "xt2", _p3())
            xt_b3 = _splitk(xTb[:, 3], QC, "xt3", _p3())
            wqk1_kt = _splitk(wqk[:, 1], DG, "wqk1", _p2())
            xts = (xt_b0, xt_b1, xt_b2, xt_b3)

            QT_sb = consts.tile([128, 2, S], BF16)
            KT_sb = consts.tile([128, 2, S], BF16)
            v_sb = consts.tile([128, ST_TILES, VAUG], BF16)

            # Filler units are (matmul-emit, evac-emit) pairs. Within a
            # step the emission order is: filler MMs, scores+exp,
            # ctx(i-1), filler evacs. Engine queues are in-order, so an
            # evac whose producing matmuls haven't retired would
            # head-of-line block the latency-critical exp op behind it;
            # with MMs at the head of the same step's TE queue, the evac
            # deps are long satisfied by the time the evac is queued.
            _acc = {}

            # ---- m=0 projection for one 512-seq block ----
            # wqk m0 layout per kt: Q cols 0:128, K cols 128:256.
            def m0_mm(b, which, warm=False, kts=range(KT_TILES)):
                col0 = 0 if which == "q" else 128
                if 0 in kts:
                    acc = ps.tile([128, QC], F32, tag="a",
                                  name=f"m0{which}{b}")
                    _acc[("m0", which, b)] = acc
                else:
                    acc = _acc[("m0", which, b)]
                if warm:
                    # p-state warm-up: hold the PE busy at FULL array
                    # utilization through the ~11us DMA load phase so
                    # the clock is ramped when real work arrives;
                    # garbage is erased by kt=0's start=True.
                    # count sized to span until block 0's last DMA slice
                    # lands (~15.5us): the PE must never idle before
                    # Kb0 or the p-state drops for most of pass 0.
                    for _ in range(37):
                        nc.tensor.matmul(
                            acc[:, 0:256], warm_sb[:, 0:128],
                            warm_sb[:, 128:384],
                            start=True, stop=True,
                        )
                for kt in kts:
                    nc.tensor.matmul(
                        acc[:],
                        wqk0_kt(kt)[:, col0:col0 + 128],
                        xts[b](kt),
                        start=(kt == 0), stop=(kt == KT_TILES - 1),
                    )

            def m0_evac(b, which):
                acc = _acc.pop(("m0", which, b))
                if which == "k":
                    nc.scalar.activation(
                        KT_sb[:, 0, b * QC:(b + 1) * QC], acc[:],
                        mybir.ActivationFunctionType.Identity,
                        bias=bqk_sb[:, 2:3],
                    )
                else:
                    nc.vector.tensor_scalar_add(
                        QT_sb[:, 0, b * QC:(b + 1) * QC], acc[:],
                        bqk_sb[:, 0:1],
                    )

            def v_mm(st):
                psv = ps.tile([128, QC], F32, tag="a", name="psv")
                _acc[("v", st)] = psv
                blk, sub = st // 4, st % 4
                for kt in range(KT_TILES):
                    nc.tensor.matmul(
                        psv[:, 0:VAUG],
                        xts[blk](kt)[:, sub * 128:(sub + 1) * 128],
                        wv_kt(kt),
                        start=(kt == 0), stop=False,
                    )
                nc.tensor.matmul(
                    psv[:, 0:VAUG], ones_sb[:, 0:128], bvaug_sb[:, :],
                    start=False, stop=True,
                )

            def v_evac(st):
                psv = _acc.pop(("v", st))
                nc.vector.tensor_copy(out=v_sb[:, st, :], in_=psv[:, 0:VAUG])

            # m=1 projection: 8-kt sweep split into 2-kt quarters
            # spread over 4 steps (an 8-matmul burst in one step starves
            # the exp pipeline for ~2us).
            def m1_mm(wcol, sc, quarter):
                if quarter == 0:
                    acc = ps.tile([128, QC], F32, tag="a",
                                  name=f"m1_{wcol}_{sc}")
                    _acc[("m1", wcol, sc)] = acc
                else:
                    acc = _acc[("m1", wcol, sc)]
                for kt in range(quarter * 2, quarter * 2 + 2):
                    nc.tensor.matmul(
                        acc[:],
                        wqk1_kt(kt)[:, wcol:wcol + 128],
                        xts[sc](kt),
                        start=(kt == 0), stop=(kt == KT_TILES - 1),
                    )

            def m1_evac(dst_sb, wcol, bcol, sc):
                acc = _acc.pop(("m1", wcol, sc))
                nc.vector.tensor_scalar_add(
                    dst_sb[:, 1, sc * QC:(sc + 1) * QC], acc[:],
                    bqk_sb[:, bcol:bcol + 1],
                )

            # ---- head: warmup + block-0 m0 only; scores(0) fires as
            # early as possible, V0/V1 become step-1 units (ctx(0) is
            # emitted at step 2 under the lag-2 pipeline).
            m0_mm(0, "k", warm=True)
            m0_evac(0, "k")
            m0_mm(0, "q")
            m0_evac(0, "q")

            # ---- filler schedule: {step: [(mm_fn, evac_fn), ...]} ----
            # At most 2 units per step (tag "a" has 2 slots; a unit's
            # psum lives only phase1->phase4 of its step).
            F = {}

            def unit(step, mm, evac):
                F.setdefault(step, []).append((mm, evac))

            def v_unit(step, st):
                unit(step, lambda: v_mm(st), lambda: v_evac(st))

            # V(st) must land before ctx(st) at step st+2; K-m0 block b
            # before scores kt=4b at step 4b; Q-m0 block j before pass
            # (0,j) at step 16j.
            def m0_halves(s0, b, which):
                unit(s0, lambda: m0_mm(b, which, kts=range(0, 4)), None)
                unit(s0 + 1, lambda: m0_mm(b, which, kts=range(4, 8)),
                     lambda: m0_evac(b, which))

            # Slot discipline on tag "a" (2 slots): each 2-step m0-half
            # holder allocates with exactly ONE transient V unit beside
            # it (other slot) and nothing allocates during its second
            # step, so evacs always precede reallocation.
            v_unit(1, 0)
            v_unit(1, 1)
            m0_halves(2, 1, "k")   # live 2-3
            v_unit(2, 2)
            v_unit(4, 3)
            v_unit(4, 4)
            m0_halves(5, 2, "k")   # live 5-6
            v_unit(5, 5)
            v_unit(7, 6)
            v_unit(7, 7)
            m0_halves(8, 3, "k")   # live 8-9
            v_unit(8, 8)
            v_unit(10, 9)
            v_unit(10, 10)
            m0_halves(11, 1, "q")  # live 11-12
            v_unit(11, 11)
            v_unit(13, 12)
            v_unit(13, 13)
            v_unit(14, 14)
            v_unit(14, 15)
            # Qb2/Qb3 split into 4-kt halves (an 8-MM burst in one step
            # starves the exp pipeline for ~2us)
            unit(30, lambda: m0_mm(2, "q", kts=range(0, 4)), None)
            unit(31, lambda: m0_mm(2, "q", kts=range(4, 8)),
                 lambda: m0_evac(2, "q"))
            unit(42, lambda: m0_mm(3, "q", kts=range(0, 4)), None)
            unit(43, lambda: m0_mm(3, "q", kts=range(4, 8)),
                 lambda: m0_evac(3, "q"))
            # m=1 sweeps: K all before pass (1,0) at step 64; Q chunk sc
            # before pass (1,sc) at step 64+16sc. Quarters at dl..dl+3,
            # evac with the last quarter.
            for i, dl in enumerate((20, 28, 38, 48)):
                for q in range(4):
                    unit(dl + q, lambda s=i, q=q: m1_mm(128, s, q),
                         (lambda s=i: m1_evac(KT_sb, 128, 3, s))
                         if q == 3 else None)
            for i, dl in enumerate((54, 70, 86, 100)):
                for q in range(4):
                    unit(dl + q, lambda s=i, q=q: m1_mm(0, s, q),
                         (lambda s=i: m1_evac(QT_sb, 0, 1, s))
                         if q == 3 else None)

            # ---- attention: 8 passes x 16 kt steps, lag-1 pipeline ----
            passes = [(p, j) for p in range(2) for j in range(N_QC)]
            steps = [(pi, kt) for pi in range(len(passes))
                     for kt in range(ST_TILES)]
            n_steps = len(steps)

            es_tiles = {}
            ctx_tiles = {}

            def emit_scores(i):
                pi, kt = steps[i]
                p, j = passes[pi]
                # Per-engine score/es tiles: separate psum rings (sscA /
                # sscD) keep each half's WAR chain independent, and
                # separate es tiles avoid a false WAW between the ACT
                # and DVE halves.
                hh_act = i % 2          # exact-exp half alternates
                hh_dve = 1 - hh_act
                sA = ps.tile([128, QC], F32, tag="sscA", name="sscA")
                sD = ps.tile([128, QC], F32, tag="sscD", name="sscD")
                for hh, st in ((hh_act, sA), (hh_dve, sD)):
                    rows = slice(hh * 64, hh * 64 + 64)
                    nc.tensor.matmul(
                        st[:],
                        KT_sb[rows, p, kt * 128:(kt + 1) * 128],
                        QT_sb[rows, p, j * QC:(j + 1) * QC],
                        start=True, stop=True,
                    )
                esA = esp.tile([128, QC], BF16, tag="esA", name="esA")
                esD = esp.tile([128, QC], I16, tag="esD", name="esD")
                nc.scalar.activation(
                    esA[:], sA[:],
                    mybir.ActivationFunctionType.Exp, scale=0.125,
                )
                nc.vector.tensor_scalar(
                    esD[:], sD[:],
                    EXP_MUL, EXP_ADD,
                    mybir.AluOpType.mult, mybir.AluOpType.add,
                )
                es_tiles[i] = {hh_act: esA[:], hh_dve: esD[:].bitcast(BF16)}

            pending_out = []

            def emit_ctx(i):
                pi, kt = steps[i]
                p, j = passes[pi]
                if kt == 0:
                    for hh in range(2):
                        ctx_tiles[(pi, hh)] = ps.tile(
                            [65, QC], F32, tag="ctx", name="ctx"
                        )
                es = es_tiles.pop(i)
                for hh in range(2):
                    h = 2 * p + hh
                    nc.tensor.matmul(
                        ctx_tiles[(pi, hh)][:],
                        v_sb[:, kt, h * 65:(h + 1) * 65],
                        es[hh],
                        start=(kt == 0), stop=(kt == ST_TILES - 1),
                    )
                if kt == ST_TILES - 1:
                    pending_out.append((i, pi,
                                        ctx_tiles.pop((pi, 0)),
                                        ctx_tiles.pop((pi, 1))))

            def flush_out(before_step=None):
                # pass-boundary out evac: hh0 on ACT, hh1 on DVE, run in
                # parallel; emitted at the TAIL of the following step's
                # queues (after its exp ops) so the ctx-stop matmul has
                # retired and the copies neither stall the exp ops nor
                # open a bubble.
                while pending_out and (before_step is None
                                       or pending_out[0][0] < before_step):
                    _, pi, c0, c1 = pending_out.pop(0)
                    p, j = passes[pi]
                    for hh, cpsum in ((0, c0), (1, c1)):
                        h = 2 * p + hh
                        ctx_sb = outp.tile([65, QC], BF16, tag="o",
                                           name="ctx_sb")
                        if hh == 0:
                            nc.scalar.copy(out=ctx_sb[:], in_=cpsum[:])
                        else:
                            nc.vector.tensor_copy(out=ctx_sb[:],
                                                  in_=cpsum[:])
                        # alternate queues so the last pass's outputs
                        # drain in parallel instead of serializing
                        q = (nc.sync, nc.scalar)[hh]
                        q.dma_start(
                            out_raw[h * 65:(h + 1) * 65,
                                    j * QC:(j + 1) * QC],
                            ctx_sb[:],
                        )

            for i in range(n_steps):
                units = F.get(i, ())
                for mm, _ in units:          # phase 1: filler matmuls
                    if mm:
                        mm()
                emit_scores(i)               # phase 2: scores + exp
                # out-copies queue behind this step's exp ops; must be
                # emitted BEFORE emit_ctx's kt=0 alloc reuses the psum.
                flush_out(before_step=i)
                if i > 1:
                    emit_ctx(i - 2)          # phase 3 (lag-2: the
                    # scores->exp->ctx chain never stalls the PE)
                for _, evac in units:        # phase 4: filler evacs
                    if evac:
                        evac()
            emit_ctx(n_steps - 2)
            emit_ctx(n_steps - 1)
            flush_out()
    nc.compile()
    return nc


_NC_CACHE = None


def _get_nc():
    global _NC_CACHE
    if _NC_CACHE is None:
        _NC_CACHE = _build_kernel()
    return _NC_CACHE


def _prep_core_inputs(hidden_states, Wq, bq, Wk, bk, Wv, bv):
    """Host-side sharding: returns list of 8 in_maps (bf16 pre-cast)."""
    # xT [1024, 2048] -> block-major [128 p, 4 blk, 8 kt, 512 s]
    xTbs = [
        np.ascontiguousarray(
            hidden_states[b].T.reshape(KT_TILES, 128, NBLK, QC)
            .transpose(1, 2, 0, 3)
        ).astype(_BF16)
        for b in range(B)
    ]
    in_maps = []
    for c in range(N_CORES):
        b, g = divmod(c, GROUPS)
        cs = slice(g * DG, (g + 1) * DG)
        wq_g = Wq[:, cs]
        wk_g = Wk[:, cs]
        wv_g = Wv[:, cs]
        bq_g, bk_g, bv_g = bq[cs], bk[cs], bv[cs]

        wv_aug = np.zeros((HIDDEN, VAUG), dtype=np.float32)
        bv_aug = np.zeros((1, VAUG), dtype=np.float32)
        for h in range(HG):
            wv_aug[:, h * 65:h * 65 + 64] = wv_g[:, h * 64:(h + 1) * 64]
            bv_aug[0, h * 65:h * 65 + 64] = bv_g[h * 64:(h + 1) * 64]
            bv_aug[0, h * 65 + 64] = 1.0

        bqk = np.stack(
            [bq_g[:128], bq_g[128:], bk_g[:128], bk_g[128:]], axis=1
        ).astype(np.float32)

        in_maps.append(
            {
                "xTb": xTbs[b],
                # partition-major SBUF image [128, 2, 8, 256]
                "wqk": np.ascontiguousarray(
                    np.stack([
                        np.concatenate(
                            [wq_g[:, m * 128:(m + 1) * 128],
                             wk_g[:, m * 128:(m + 1) * 128]], 1
                        ).reshape(KT_TILES, 128, DG).transpose(1, 0, 2)
                        for m in range(2)
                    ], axis=1)
                ).astype(_BF16),
                # partition-major SBUF image [128, 8, 260]
                "wv": np.ascontiguousarray(
                    wv_aug.reshape(KT_TILES, 128, VAUG).transpose(1, 0, 2)
                ).astype(_BF16),
                "bqk": np.ascontiguousarray(bqk),
                "bv_aug": bv_aug.astype(_BF16),
            }
        )
    return in_maps


def _unshard(results):
    out = np.empty((B, S, HIDDEN), dtype=np.float32)
    for c in range(N_CORES):
        b, g = divmod(c, GROUPS)
        raw = np.asarray(results[c]["out_raw"], dtype=np.float32)
        for h in range(HG):
            ctx = raw[h * 65:h * 65 + 64]          # [64, S]
            sums = raw[h * 65 + 64]                # [S]
            col0 = g * DG + h * HEAD
            out[b, :, col0:col0 + HEAD] = (ctx / sums).T
    return out


def kernel(**inputs):
    inputs = {k: np.asarray(v, dtype=np.float32) for k, v in inputs.items()}
    nc = _get_nc()
    in_maps = _prep_core_inputs(**inputs)
    res = run_bass_kernel_spmd(nc, in_maps, core_ids=list(range(N_CORES)))
    return _unshard(res.results)


if __name__ == "__main__":
    rng = np.random.default_rng(0)
    scale = 1.0 / np.sqrt(HIDDEN)
    ins = {
        "hidden_states": rng.standard_normal((B, S, HIDDEN), dtype=np.float32),
        "Wq": rng.standard_normal((HIDDEN, HIDDEN), dtype=np.float32) * scale,
        "bq": rng.standard_normal(HIDDEN, dtype=np.float32) * 0.01,
        "Wk": rng.standard_normal((HIDDEN, HIDDEN), dtype=np.float32) * scale,
        "bk": rng.standard_normal(HIDDEN, dtype=np.float32) * 0.01,
        "Wv": rng.standard_normal((HIDDEN, HIDDEN), dtype=np.float32) * scale,
        "bv": rng.standard_normal(HIDDEN, dtype=np.float32) * 0.01,
    }
    out = kernel(**ins)

    def ref(x, Wq, bq, Wk, bk, Wv, bv):
        q = (x @ Wq + bq).reshape(B, S, NUM_HEADS, HEAD).transpose(0, 2, 1, 3)
        k = (x @ Wk + bk).reshape(B, S, NUM_HEADS, HEAD).transpose(0, 2, 1, 3)
        v = (x @ Wv + bv).reshape(B, S, NUM_HEADS, HEAD).transpose(0, 2, 1, 3)
        s = np.einsum("bhqd,bhkd->bhqk", q, k) / np.sqrt(HEAD)
        s = s - s.max(-1, keepdims=True)
        p = np.exp(s)
        p /= p.sum(-1, keepdims=True)
        c = np.einsum("bhqk,bhkd->bhqd", p, v)
        return c.transpose(0, 2, 1, 3).reshape(B, S, HIDDEN)

    exp = ref(
        ins["hidden_states"].astype(np.float64),
        ins["Wq"].astype(np.float64), ins["bq"].astype(np.float64),
        ins["Wk"].astype(np.float64), ins["bk"].astype(np.float64),
        ins["Wv"].astype(np.float64), ins["bv"].astype(np.float64),
    )
    print("L2 rel err:", np.linalg.norm(out - exp) / np.linalg.norm(exp))
    print("max abs err:", np.abs(out - exp).max())
